# revision 29
# baseline (speedup 1.0000x reference)
"""Multi-head attention Trainium2 kernel (8-core SPMD, head-parallel).

Problem: nn_MultiHeadAttention (B=2, S=2048, d_model=1024, H=16, d_k=64).

Sharding: heads are split across the 8 cores (2 heads x 2 batches per core).
Each core holds the column block of W_q/W_k/W_v for its 2 heads and the
matching row block of W_o.T, computes a full [4096, 1024] partial of the
output projection, and the host sums the 8 partials (the "all-reduce").

Per-core layout strategy (everything keyed to keep the contraction dim on
SBUF partitions with contiguous DMA):
  - Host ships X.T = [1024, 4096] bf16 per input, so projection matmuls read
    both operands naturally.  Q/K are produced transposed (Q.T = [128, 4096]),
    which is exactly the layout the scores matmul wants.
  - Scores are computed transposed: S.T[k, q] = K_h Q_h.T, so the attention
    weight matrix lands with k on partitions, exp applied on eviction
    (ScalarE), and the AV matmul consumes it directly as the moving operand
    with V in natural [token, d] layout as the stationary operand.  Both
    heads' score tiles share one 2-bank PSUM tile so a single wide ACTIVATE
    handles the exp for both.
  - Softmax denominators come for free by appending a ones column to V
    (M=65): row 64 of the AV accumulator is sum_k exp(s).  Softmax
    max-subtraction is skipped: scores are ~N(0,1) so exp never overflows,
    and softmax is shift-invariant.
  - Normalization happens per chunk at AV eviction: denominator row to
    partition 0 (shift DMA), reciprocal, GPSIMD partition-broadcast, multiply
    fused with the PSUM eviction.  The output projection runs per chunk.
  - The two batches are pipelined: batch-1 projections are emitted between
    batch-0 attention chunks.
"""

import os
import sys

sys.path.insert(0, "/opt/trn_rl_repo")

import numpy as np
import ml_dtypes

import concourse.bass as bass
import concourse.mybir as mybir
import concourse.tile as tile
import concourse.bacc as bacc
from concourse.bass_utils import run_bass_kernel_spmd

BF16 = mybir.dt.bfloat16
F32 = mybir.dt.float32
NP_BF16 = ml_dtypes.bfloat16

B, S, D = 2, 2048, 1024
H, DK = 16, 64
T = B * S            # 4096 tokens
N_CORES = 8
HPC = H // N_CORES   # heads per core = 2
HD = HPC * DK        # 128 per-core head dims
KT = D // 128        # 8 contraction tiles for projections
TCH = 512            # token chunk for projections
QC = 512             # q chunk for attention
NKT = S // 128       # 16 k tiles per batch
VEXT_W = 192         # [V_A(64)|1|pad(31)][V_B(64)|1|pad(31)] (32-aligned)
SCALE = 1.0 / np.sqrt(DK)
EXP = mybir.ActivationFunctionType.Exp


def build_nc():
    nc = bacc.Bacc("TRN2", target_bir_lowering=False, debug=False,
                   num_devices=N_CORES)

    xq_t = nc.dram_tensor("xq_t", [D, T], BF16, kind="ExternalInput").ap()
    xk_t = nc.dram_tensor("xk_t", [D, T], BF16, kind="ExternalInput").ap()
    xv_t = nc.dram_tensor("xv_t", [D, T], BF16, kind="ExternalInput").ap()
    wq_t = nc.dram_tensor("wq_t", [D, HD], BF16, kind="ExternalInput").ap()
    wk_t = nc.dram_tensor("wk_t", [D, HD], BF16, kind="ExternalInput").ap()
    wv_t = nc.dram_tensor("wv_t", [D, HD], BF16, kind="ExternalInput").ap()
    wo = nc.dram_tensor("wo", [HD, D], BF16, kind="ExternalInput").ap()
    out_p = nc.dram_tensor("out_p", [T, D], F32, kind="ExternalOutput").ap()

    with tile.TileContext(nc) as tc, \
         tc.tile_pool(name="persist", bufs=1) as persist, \
         tc.tile_pool(name="xwp", bufs=16) as xwp, \
         tc.tile_pool(name="vst", bufs=2) as vst, \
         tc.tile_pool(name="pp", bufs=8) as pp, \
         tc.tile_pool(name="ev", bufs=3) as ev, \
         tc.tile_pool(name="op", bufs=4) as op, \
         tc.tile_pool(name="sc_ps", bufs=2, space="PSUM") as sc_ps, \
         tc.tile_pool(name="av_ps", bufs=1, space="PSUM") as av_ps, \
         tc.tile_pool(name="pj_ps", bufs=2, space="PSUM") as pj_ps:

        def ptile(shape, dtype, name):
            return persist.tile(shape, dtype, name=name, tag=name)

        wq_sb = ptile([128, KT * HD], BF16, "wq_sb")
        wk_sb = ptile([128, KT * HD], BF16, "wk_sb")
        wv_sb = ptile([128, KT * HD], BF16, "wv_sb")
        wo_sb = ptile([128, D], BF16, "wo_sb")
        qt_sb = ptile([128, T], BF16, "qt_sb")
        kt_sb = ptile([128, T], BF16, "kt_sb")
        vt_sb = ptile([128, T], BF16, "vt_sb")
        vext_sb = ptile([128, (T // 128) * VEXT_W], BF16, "vext_sb")
        ot_sb = ptile([128, T], BF16, "ot_sb")

        # ---- weight loads + constants (single DMA each) ------------------
        for w_sb, w_dram in ((wq_sb, wq_t), (wk_sb, wk_t), (wv_sb, wv_t)):
            nc.sync.dma_start(
                w_sb[:].rearrange("p (j d) -> p j d", j=KT),
                w_dram.rearrange("(j p) d -> p j d", p=128))
        nc.sync.dma_start(wo_sb[:], wo[:])
        vext_g = vext_sb[:].rearrange("p (i two c) -> p i two c", two=2, c=96)
        nc.vector.memset(vext_g[:, :, :, 64:65], 1.0)
        # load the exp ACT table set while the head DMAs stream
        warm = ev.tile([1, 32], F32, tag="warm")
        nc.vector.memset(warm[:], 0.0)
        nc.scalar.activation(warm[:], warm[:], EXP)

        # ---- emission units ----------------------------------------------
        def proj_unit(dst, w_sb, x_dram, b, j_outer=False):
            """Project one batch's 2048 tokens for one of K/V/Q.

            j_outer=True uses 4 PSUM accumulators (both sc slots) with the
            contraction outermost, so each x tile is released after 4 matmuls
            and the DMA stream never stalls on pool slots.  Only safe when
            attention is not competing for the sc slots (batch-0 head).
            """
            bc = b * S
            if j_outer:
                pa0 = sc_ps.tile([128, 2 * TCH], F32, name="pa0", tag="sc")
                pa1 = sc_ps.tile([128, 2 * TCH], F32, name="pa1", tag="sc")
                accs = [pa0[:, 0:TCH], pa0[:, TCH:2 * TCH],
                        pa1[:, 0:TCH], pa1[:, TCH:2 * TCH]]
                for j in range(KT):
                    xw = xwp.tile([128, S], BF16, name=f"xw{j}", tag="xw")
                    nc.sync.dma_start(xw[:],
                                      x_dram[j * 128:(j + 1) * 128, bc:bc + S])
                    for t4 in range(S // TCH):
                        nc.tensor.matmul(accs[t4],
                                         lhsT=w_sb[:, j * HD:(j + 1) * HD],
                                         rhs=xw[:, t4 * TCH:(t4 + 1) * TCH],
                                         start=(j == 0), stop=(j == KT - 1))
                for t4 in range(S // TCH):
                    nc.vector.tensor_copy(
                        dst[:, bc + t4 * TCH:bc + (t4 + 1) * TCH], accs[t4])
                return
            xws = []
            for j in range(KT):
                xw = xwp.tile([128, S], BF16, name=f"xw{j}", tag="xw")
                nc.sync.dma_start(xw[:],
                                  x_dram[j * 128:(j + 1) * 128, bc:bc + S])
                xws.append(xw)
            for t4 in range(S // TCH):
                ps = pj_ps.tile([128, TCH], F32, name="ps", tag="pj")
                for j in range(KT):
                    nc.tensor.matmul(ps[:], lhsT=w_sb[:, j * HD:(j + 1) * HD],
                                     rhs=xws[j][:, t4 * TCH:(t4 + 1) * TCH],
                                     start=(j == 0), stop=(j == KT - 1))
                nc.vector.tensor_copy(
                    dst[:, bc + t4 * TCH:bc + (t4 + 1) * TCH], ps[:])

        # ---- background generators (one quantum per yield) ---------------
        def proj_gen(dst, w_sb, x_dram, b):
            """Batch-1 projection: loads prefetched up-front, one matmul per
            quantum, pj-pool accumulators (no contention with scores)."""
            bc = b * S
            xws = []
            for j in range(KT):
                xw = xwp.tile([128, S], BF16, name=f"xw{j}", tag="xw")
                nc.sync.dma_start(xw[:],
                                  x_dram[j * 128:(j + 1) * 128, bc:bc + S])
                xws.append(xw)
            for t4 in range(S // TCH):
                ps = pj_ps.tile([128, TCH], F32, name="ps", tag="pj")
                for j in range(KT):
                    nc.tensor.matmul(ps[:], lhsT=w_sb[:, j * HD:(j + 1) * HD],
                                     rhs=xws[j][:, t4 * TCH:(t4 + 1) * TCH],
                                     start=(j == 0), stop=(j == KT - 1))
                    yield
                nc.vector.tensor_copy(
                    dst[:, bc + t4 * TCH:bc + (t4 + 1) * TCH], ps[:])
                yield

        def vtrans_gen(b):
            """V.T tiles -> v_ext natural layout (xbar DMA + strided copy)."""
            for il in range(NKT):
                i = b * NKT + il
                icol = i * 128
                base = i * VEXT_W
                vn = vst.tile([128, 128], BF16, name="vn", tag="vn")
                nc.sync.dma_start_transpose(vn[:], vt_sb[:, icol:icol + 128])
                dstv = vext_sb[:, base:base + VEXT_W].rearrange(
                    "p (two c) -> p two c", c=96)[:, :, 0:64]
                nc.vector.tensor_copy(
                    dstv, vn[:].rearrange("p (two c) -> p two c", c=64))
                yield

        def stage5_gen(b, qc):
            """Output projection for one chunk's token tiles."""
            col = b * S + qc * QC
            for tl in range(QC // 128):
                tcol = col + tl * 128
                ost = op.tile([128, D], F32, tag="o")
                for c in range(D // 512):
                    pso = pj_ps.tile([128, 512], F32, name="pso", tag="pj")
                    nc.tensor.matmul(pso[:],
                                     lhsT=ot_sb[:, tcol:tcol + 128],
                                     rhs=wo_sb[:, c * 512:(c + 1) * 512],
                                     start=True, stop=True)
                    nc.vector.tensor_copy(ost[:, c * 512:(c + 1) * 512],
                                          pso[:])
                    yield
                nc.gpsimd.dma_start(out_p[tcol:tcol + 128, :], ost[:])
                yield

        def vtrans(b):
            for _ in vtrans_gen(b):
                pass

        def attn(b, qc, background):
            """One attention chunk; interleaves background quanta between
            i-steps so in-order engines never see long foreign blocks."""
            col = b * S + qc * QC
            accA = av_ps.tile([65, QC], F32, name="accA", tag="avA")
            accB = av_ps.tile([65, QC], F32, name="accB", tag="avB")
            pend = {}
            for i in range(NKT + 1):
                if i < NKT:
                    # scores + exp run one step ahead of the AV chain so the
                    # exp stream keeps flowing while the accumulator slot of
                    # the previous chunk is still draining
                    kcol = b * S + i * 128
                    sc = sc_ps.tile([128, 2 * QC], F32, name="sc", tag="sc")
                    nc.tensor.matmul(sc[:, 0:QC],
                                     lhsT=kt_sb[0:64, kcol:kcol + 128],
                                     rhs=qt_sb[0:64, col:col + QC],
                                     start=True, stop=True)
                    nc.tensor.matmul(sc[:, QC:2 * QC],
                                     lhsT=kt_sb[64:128, kcol:kcol + 128],
                                     rhs=qt_sb[64:128, col:col + QC],
                                     start=True, stop=True)
                    p = pp.tile([128, 2 * QC], BF16, tag="p")
                    nc.scalar.activation(p[:], sc[:], EXP, scale=float(SCALE))
                    pend[i] = p
                if i >= 1:
                    ii = i - 1
                    p = pend.pop(ii)
                    vbase = (b * NKT + ii) * VEXT_W
                    nc.tensor.matmul(accA[:],
                                     lhsT=vext_sb[:, vbase:vbase + 65],
                                     rhs=p[:, 0:QC],
                                     start=(ii == 0), stop=(ii == NKT - 1))
                    nc.tensor.matmul(accB[:],
                                     lhsT=vext_sb[:, vbase + 96:vbase + 161],
                                     rhs=p[:, QC:2 * QC],
                                     start=(ii == 0), stop=(ii == NKT - 1))
                for _ in range(4):
                    if not background:
                        break
                    try:
                        next(background[0])
                    except StopIteration:
                        background.pop(0)
            # psum rows 0-63 = O_unnorm, row 64 = softmax denominator
            for hh, ps in (("A", accA), ("B", accB)):
                dt = ev.tile([65, QC], F32, tag="dt" + hh)
                nc.vector.reciprocal(dt[64:65, :], ps[64:65, :])
                dn = ev.tile([1, QC], F32, tag="dn" + hh)
                nc.gpsimd.dma_start(dn[0:1, :], dt[64:65, :])
                rb = ev.tile([64, QC], F32, tag="rb" + hh)
                nc.gpsimd.partition_broadcast(rb[0:64, :], dn[0:1, :],
                                              channels=64)
                if hh == "A":
                    nc.vector.tensor_mul(ot_sb[0:64, col:col + QC],
                                         ps[0:64, :], rb[0:64, :])
                else:
                    bt = ev.tile([64, QC], BF16, tag="bt")
                    nc.vector.tensor_mul(bt[0:64, :], ps[0:64, :], rb[0:64, :])
                    nc.gpsimd.dma_start(ot_sb[64:128, col:col + QC],
                                        bt[0:64, :])

        # ---- pipelined emission ------------------------------------------
        # Head: batch-0 projections (K and V j-outer on the sc slots, Q on
        # the pj slots), then attention with all remaining work drip-fed
        # between attention i-steps.
        proj_unit(kt_sb, wk_sb, xk_t, 0, j_outer=True)
        proj_unit(qt_sb, wq_sb, xq_t, 0)
        proj_unit(vt_sb, wv_sb, xv_t, 0, j_outer=True)
        vtrans(0)
        background = [
            proj_gen(kt_sb, wk_sb, xk_t, 1),
            proj_gen(vt_sb, wv_sb, xv_t, 1),
            vtrans_gen(1),
            proj_gen(qt_sb, wq_sb, xq_t, 1),
        ]
        attn(0, 0, background)
        background.append(stage5_gen(0, 0))
        attn(0, 1, background)
        background.append(stage5_gen(0, 1))
        attn(0, 2, background)
        background.append(stage5_gen(0, 2))
        attn(0, 3, background)
        background.append(stage5_gen(0, 3))
        attn(1, 0, background)
        background.append(stage5_gen(1, 0))
        attn(1, 1, background)
        background.append(stage5_gen(1, 1))
        attn(1, 2, background)
        background.append(stage5_gen(1, 2))
        attn(1, 3, background)
        for g in background:
            for _ in g:
                pass
        for _ in stage5_gen(1, 3):
            pass

    nc.compile()
    return nc


def make_in_maps(query, key, value, W_q, W_k, W_v, W_o):
    def xT(x):
        return np.ascontiguousarray(
            np.asarray(x, np.float32).reshape(T, D).astype(NP_BF16).T)

    xq, xk, xv = xT(query), xT(key), xT(value)
    W_q = np.asarray(W_q, np.float32)
    W_k = np.asarray(W_k, np.float32)
    W_v = np.asarray(W_v, np.float32)
    W_o = np.asarray(W_o, np.float32)
    in_maps = []
    for m in range(N_CORES):
        r = slice(m * HD, (m + 1) * HD)
        in_maps.append({
            "xq_t": xq, "xk_t": xk, "xv_t": xv,
            "wq_t": np.ascontiguousarray(W_q[r, :].T).astype(NP_BF16),
            "wk_t": np.ascontiguousarray(W_k[r, :].T).astype(NP_BF16),
            "wv_t": np.ascontiguousarray(W_v[r, :].T).astype(NP_BF16),
            "wo": np.ascontiguousarray(W_o[:, r].T).astype(NP_BF16),
        })
    return in_maps


_NC_CACHE = None


def get_nc():
    global _NC_CACHE
    if _NC_CACHE is None:
        _NC_CACHE = build_nc()
    return _NC_CACHE


def kernel(query, key, value, W_q, W_k, W_v, W_o):
    nc = get_nc()
    in_maps = make_in_maps(query, key, value, W_q, W_k, W_v, W_o)
    res = run_bass_kernel_spmd(nc, in_maps, core_ids=list(range(N_CORES)))
    acc = np.zeros((T, D), np.float32)
    for m in range(N_CORES):
        acc += res.results[m]["out_p"]
    return acc.reshape(B, S, D)
